# revision 12
# baseline (speedup 1.0000x reference)
"""DigitCaps routing kernel for TRN2 (8 NeuronCores, SPMD data-parallel over batch).

Problem: inputs [64, 4096, 8] f32, W [4096, 10, 8, 16] f32.
  u_hat[b,i,n,d] = sum_p inputs[b,i,p] * W[i,n,p,d]
  3 dynamic-routing iterations (softmax over n, weighted sum over i, squash,
  agreement update), output v [64, 10, 1, 16] f32.

Sharding: batch 64 -> 8 cores x 8 samples. W replicated (streamed once per core).
All on-device data is f32: the routing fixed-point amplifies input rounding by
~100-1000x, so fp16/bf16 anywhere fails accuracy (verified numerically).

Per-core device layout (sigma = i'*8 + b, where i' = i mod 16, b = local sample):
  U  [128=(i'*8+b), 256 chunks * 160]  f32   u_hat, chunk k holds i in [16k,16k+16)
Production: per chunk one matmul  lhsT = Xall_k [128=(i'*8+p), 128=(i'*8+b)]
  (block-diagonal x, built on-chip from xpk*xmask), rhs = W2_k [128, 160].
Routing (v2 redesign vs the DMA-extract baseline):
  - logits are linear in u: b_t = u . (v_0+..+v_{t-1}), so no logit state is
    kept; each iteration computes L fresh from the running v-sum (saves the
    L accumulate pass + memset).
  - per group of 16 chunks: pr = U*vf (DVE / GPSIMD alternating), L =
    reduce_d(pr) (DVE), ee = exp(L) (ACT), den/rr (DVE), ca = ee x (rr*maskb)
    (GPSIMD), then 16 s-matmuls (PE) accumulate [80,160] PSUM.
  - s extraction: diag-mask multiply + sel8 fold matmul (replaces 10 serial
    DMAs per iteration).
  - squash via sign-trick: v = sign(s)*s^2/(1+s^2)  (sqrt(s^2+eps) ~ |s|,
    abs err <= sqrt(eps)/2 ~ 1.6e-4; all-DVE, no ACT table switches).
  - PSUM->SBUF U copies mostly on ACT (frees DVE for xab build in the
    DMA-bound production window).
"""

from contextlib import ExitStack

import numpy as np

import concourse.bass as bass
import concourse.tile as tile
from concourse import bacc, mybir
from concourse.tile import TileContext

AF = mybir.ActivationFunctionType
ALU = mybir.AluOpType

N_CORES = 8
B_FULL = 64
I_FULL = 4096
P_DIM = 8          # Din
N_CAP = 10
D_CAP = 16
ND = N_CAP * D_CAP  # 160
EPS = 1e-7
ROUTING_ITERS = 3

F32 = mybir.dt.float32

# engine-assignment experiment knobs
PR_GPS_MOD = 0     # 0 = pr always DVE; m>0 = every m-th group's pr on GPSIMD
CA_GPS = False     # ca build on GPSIMD instead of DVE
COPY_DVE_MOD = 4   # every m-th psum->U copy group on DVE, rest ACT


def build_nc(I_dim=I_FULL, b_shard=8, phases="all", repeat=1,
             pr_gps_mod=PR_GPS_MOD, ca_gps=CA_GPS):
    """Build the single-core Bass program (SPMD: same program on all cores)."""
    CH = I_dim // 16          # chunks of 16 capsules
    SUPC = 4                  # chunks per DMA superchunk
    SUP = CH // SUPC
    GRP = min(16, CH)         # chunks per group for the routing pipeline
    NG = CH // GRP

    nc = bacc.Bacc(dynamic_dma_scratch_size=1024)

    w2_d = nc.dram_tensor("w2", [CH * 128, ND], F32, kind="ExternalInput")
    # xpk[q=(i'*8+p), k*8+b] = x[b, 16k+i', p] — compact per-chunk x operand
    xpk_d = nc.dram_tensor("xpk", [128, CH * 8], F32, kind="ExternalInput")
    # xmask[q, m=(i2*8+b)] = 1 if i2 == q//8 else 0 (block-diagonal selector)
    xmask_d = nc.dram_tensor("xmask", [128, 128], F32, kind="ExternalInput")
    mask0_d = nc.dram_tensor("mask0", [128, 8], F32, kind="ExternalInput")
    maskb_d = nc.dram_tensor("maskb", [128, 8], F32, kind="ExternalInput")
    e8_d = nc.dram_tensor("e8", [8, 128], F32, kind="ExternalInput")
    dmask_d = nc.dram_tensor("dmask", [80, ND], F32, kind="ExternalInput")
    sel8_d = nc.dram_tensor("sel8", [80, 8], F32, kind="ExternalInput")
    out_d = nc.dram_tensor("out", [b_shard, ND], F32, kind="ExternalOutput")

    with TileContext(nc) as tc, ExitStack() as ctx:
        # ---- pools ----
        pU = ctx.enter_context(tc.tile_pool(name="U", bufs=1))
        pconst = ctx.enter_context(tc.tile_pool(name="const", bufs=1))
        # big shared rotating pool: w2b (production) and pr/ca (routing)
        pbig = ctx.enter_context(tc.tile_pool(name="big", bufs=3))
        pxa = ctx.enter_context(tc.tile_pool(name="xa", bufs=2))
        psm = ctx.enter_context(tc.tile_pool(name="sm", bufs=2))
        psmall = ctx.enter_context(tc.tile_pool(name="small", bufs=1))
        ppsP = ctx.enter_context(tc.tile_pool(name="psP", bufs=5, space="PSUM"))
        ppsS = ctx.enter_context(tc.tile_pool(name="psS", bufs=1, space="PSUM"))
        ppsV = ctx.enter_context(tc.tile_pool(name="psV", bufs=1, space="PSUM"))
        ppsE = ctx.enter_context(tc.tile_pool(name="psE", bufs=1, space="PSUM"))

        # ---- persistent tiles ----
        U = pU.tile([128, CH * ND], F32)
        xpk_sb = pconst.tile([128, CH * 8], F32)
        xmask_sb = pconst.tile([128, 128], F32)
        mask0_sb = pconst.tile([128, 8], F32)
        maskb_sb = pconst.tile([128, 8], F32)
        e8_sb = pconst.tile([8, 128], F32)
        dmask_sb = pconst.tile([80, ND], F32)
        sel8_sb = pconst.tile([80, 8], F32)

        nc.sync.dma_start(xpk_sb[:], xpk_d[:])
        nc.sync.dma_start(xmask_sb[:], xmask_d[:])
        nc.sync.dma_start(mask0_sb[:], mask0_d[:])
        nc.sync.dma_start(maskb_sb[:], maskb_d[:])
        nc.sync.dma_start(e8_sb[:], e8_d[:])
        nc.sync.dma_start(dmask_sb[:], dmask_d[:])
        nc.sync.dma_start(sel8_sb[:], sel8_d[:])

        # ---- phase A: u_hat production ----
        w2_r = w2_d.rearrange("(s c p) f -> s p c f", c=SUPC, p=128)
        xpk_r = xpk_sb.rearrange("p (s c b) -> p s c b", c=SUPC, b=8)

        def produce():
          ps = None
          for s in range(SUP):
            w2b = pbig.tile([128, SUPC * ND], F32, tag="big")
            (nc.sync if s % 2 == 0 else nc.scalar).dma_start(
                w2b.rearrange("p (c f) -> p c f", c=SUPC), w2_r[s])
            xab = pxa.tile([128, SUPC * 128], F32)
            nc.vector.tensor_tensor(
                xab.rearrange("p (c i b) -> p c i b", c=SUPC, b=8),
                xpk_r[:, s].unsqueeze(2).to_broadcast([128, SUPC, 16, 8]),
                xmask_sb.rearrange("p (i b) -> p i b", b=8)
                    .unsqueeze(1).to_broadcast([128, SUPC, 16, 8]),
                ALU.mult,
            )
            for c in range(SUPC):
                k = s * SUPC + c
                j = k % 3
                if j == 0:
                    ps = ppsP.tile([128, 3 * ND], F32)
                nc.tensor.matmul(
                    ps[:, j * ND:(j + 1) * ND],
                    xab[:, c * 128:(c + 1) * 128],
                    w2b[:, c * ND:(c + 1) * ND],
                    start=True, stop=True,
                )
                if j == 2 or k == CH - 1:
                    lo = k - j
                    if (k // 3) % COPY_DVE_MOD == COPY_DVE_MOD - 1:
                        nc.vector.tensor_copy(
                            U[:, lo * ND:(k + 1) * ND], ps[:, 0:(j + 1) * ND])
                    else:
                        nc.scalar.copy(
                            U[:, lo * ND:(k + 1) * ND], ps[:, 0:(j + 1) * ND])

        # ---- helpers ----
        def squash(s_ap, out_tag="sq_v"):
            """v = sign(s)*s^2/(1+s^2)  (elementwise, [8,160], all-DVE)."""
            s_sb = psmall.tile([8, ND], F32, tag="sq_s")
            nc.vector.tensor_scalar_mul(s_sb[:], s_ap, 1.0)
            sq = psmall.tile([8, ND], F32, tag="sq_sq")
            nc.vector.tensor_mul(sq[:], s_sb[:], s_sb[:])
            dn = psmall.tile([8, ND], F32, tag="sq_dn")
            nc.vector.tensor_scalar_add(dn[:], sq[:], 1.0)
            nc.vector.reciprocal(dn[:], dn[:])
            tt = psmall.tile([8, ND], F32, tag="sq_t")
            nc.vector.tensor_mul(tt[:], sq[:], dn[:])
            gg = psmall.tile([8, ND], F32, tag="sq_dn")
            nc.vector.tensor_scalar(gg[:], s_sb[:], 0.0, None, ALU.is_ge)
            v_sb = psmall.tile([8, ND], F32, tag=out_tag)
            nc.vector.scalar_tensor_tensor(
                v_sb[:], gg[:], 2.0, tt[:], ALU.mult, ALU.mult)
            nc.vector.tensor_tensor(v_sb[:], v_sb[:], tt[:], ALU.subtract)
            return v_sb

        def s_uniform():
            """s0 = 0.1 * sum_i u_hat -> [8, 160] psum accumulation (PE;
            hides under the DMA-bound production window)."""
            s0_ps = ppsS.tile([8, ND], F32, tag="s_acc")
            for k in range(CH):
                nc.tensor.matmul(
                    s0_ps[:], mask0_sb[:], U[:, k * ND:(k + 1) * ND],
                    start=(k == 0), stop=(k == CH - 1),
                )
            return s0_ps

        def broadcast_v(v_sb):
            """v [8,160] -> vf [128,160] (replicated per sample block)."""
            vf_ps = ppsV.tile([128, ND], F32)
            nc.tensor.matmul(vf_ps[:], e8_sb[:], v_sb[:], start=True, stop=True)
            vf = psmall.tile([128, ND], F32, tag="vf")
            nc.vector.tensor_copy(vf[:], vf_ps[:])
            return vf

        def s_iteration(vf):
            """One routing iteration: logits from vf (= running v-sum),
            softmax, masked weights, s-matmuls. Groups are processed in
            PAIRS: per-group pr + d-reduce feed one pair-wide exp / den /
            rm / ca (fewer, larger DVE ops on the serial chain)."""
            s_ps = ppsS.tile([80, ND], F32, tag="s_acc")
            assert NG % 2 == 0
            G2 = 2 * GRP
            for h in range(NG // 2):
                Lp = psm.tile([128, G2 * N_CAP], F32, tag="Lg")
                for gi in range(2):
                    g = 2 * h + gi
                    u_g = U[:, g * GRP * ND:(g + 1) * GRP * ND]
                    pr = pbig.tile([128, GRP * ND], F32, tag="big")
                    nc.vector.tensor_tensor(
                        pr.rearrange("p (k f) -> p k f", k=GRP),
                        u_g.rearrange("p (k f) -> p k f", k=GRP),
                        vf[:].unsqueeze(1).to_broadcast([128, GRP, ND]),
                        ALU.mult,
                    )
                    nc.vector.tensor_reduce(
                        Lp[:, gi * GRP * N_CAP:(gi + 1) * GRP * N_CAP],
                        pr.rearrange("p (a d) -> p a d", d=D_CAP),
                        axis=mybir.AxisListType.X, op=ALU.add)
                # Lp := exp(Lp) in place (ACT)
                nc.scalar.activation(Lp[:], Lp[:], AF.Exp)
                den = psm.tile([128, G2], F32, tag="den")
                nc.vector.tensor_reduce(
                    den[:], Lp.rearrange("p (k n) -> p k n", n=N_CAP),
                    axis=mybir.AxisListType.X, op=ALU.add)
                nc.vector.reciprocal(den[:], den[:])
                # rm[s,(k,b')] = maskb[s,b'] * (1/den[s,k])
                rm = psm.tile([128, G2 * 8], F32, tag="rm")
                nc.vector.tensor_tensor(
                    rm.rearrange("p (k b) -> p k b", b=8),
                    maskb_sb[:].unsqueeze(1).to_broadcast([128, G2, 8]),
                    den[:].unsqueeze(2).to_broadcast([128, G2, 8]),
                    ALU.mult,
                )
                ca = pbig.tile([128, G2 * 80], F32, tag="big")
                nc.vector.tensor_tensor(
                    ca.rearrange("p (k n b) -> p k n b", k=G2, b=8),
                    Lp.rearrange("p (k n) -> p k n", n=N_CAP)
                        .unsqueeze(3).to_broadcast([128, G2, N_CAP, 8]),
                    rm.rearrange("p (k b) -> p k b", b=8)
                        .unsqueeze(2).to_broadcast([128, G2, N_CAP, 8]),
                    ALU.mult,
                )
                for kk in range(G2):
                    k = h * G2 + kk
                    nc.tensor.matmul(
                        s_ps[:],
                        ca[:, kk * 80:(kk + 1) * 80],
                        U[:, k * ND:(k + 1) * ND],
                        start=(k == 0), stop=(k == CH - 1),
                    )
            return s_ps

        def extract(s_ps):
            """[80,160] psum -> [8,160] psum: diag-mask + sel8 fold matmul."""
            sm = psmall.tile([80, ND], F32, tag="sm")
            nc.vector.tensor_tensor(sm[:], s_ps[:], dmask_sb[:], ALU.mult)
            s2 = ppsE.tile([8, ND], F32, tag="sx")
            nc.tensor.matmul(s2[:], sel8_sb[:], sm[:], start=True, stop=True)
            return s2

        # ---- routing ----
        for rep in range(repeat):
            produce()
            if phases == "prod":
                v_sb = psmall.tile([8, ND], F32, tag="sq_v")
                nc.vector.tensor_copy(v_sb[:], U[0:8, 0:ND])
            elif phases == "it0":
                v_sb = squash(s_uniform()[:])
                broadcast_v(v_sb)
            elif phases == "s1":
                v0 = squash(s_uniform()[:], out_tag="sq_v0")
                v_sb = squash(extract(s_iteration(broadcast_v(v0)))[:])
            else:
                v0 = squash(s_uniform()[:], out_tag="sq_v0")
                v1 = squash(extract(s_iteration(broadcast_v(v0)))[:],
                            out_tag="sq_v1")
                vs = psmall.tile([8, ND], F32, tag="sq_vs")
                nc.vector.tensor_tensor(vs[:], v0[:], v1[:], ALU.add)
                v_sb = squash(extract(s_iteration(broadcast_v(vs)))[:])

            nc.sync.dma_start(out_d[:], v_sb[:])

    nc.compile()
    if not nc.is_finalized():
        nc.finalize()
    return nc


# ------------------------- host-side data prep -------------------------

def prep_core_inputs(x_shard, I_dim=I_FULL):
    """Per-core xpk from x_shard [8, I, 8] f32."""
    CH = I_dim // 16
    b_shard = x_shard.shape[0]
    assert b_shard == 8

    # xs[b, k, i', p] -> xpk[(i'*8+p), k*8+b]
    xs = x_shard.reshape(b_shard, CH, 16, P_DIM)
    xpk = np.ascontiguousarray(
        np.transpose(xs, (2, 3, 1, 0)).reshape(128, CH * 8))
    return {"xpk": xpk}


def prep_shared_inputs(W_np):
    # w2[(i*8+p), n*16+d] = W[i, n, p, d]
    w2 = np.ascontiguousarray(
        np.transpose(W_np, (0, 2, 1, 3)).reshape(-1, ND).astype(np.float32))

    # sigma = i'*8 + b
    # mask0[sigma, b'] = 0.1 * (b' == b(sigma))   (1/N_CAP baked in)
    # maskb[sigma, b'] = (b' == b(sigma))
    # e8[b, sigma] = (b == b(sigma))
    # xmask[q=(i'*8+p), i2*8+b] = (i2 == i')
    mask0 = np.zeros((128, 8), dtype=np.float32)
    maskb = np.zeros((128, 8), dtype=np.float32)
    e8 = np.zeros((8, 128), dtype=np.float32)
    xmask = np.zeros((128, 128), dtype=np.float32)
    for ip in range(16):
        for b in range(8):
            sig = ip * 8 + b
            mask0[sig, b] = 0.1
            maskb[sig, b] = 1.0
            e8[b, sig] = 1.0
    for ii in range(16):
        for p in range(P_DIM):
            xmask[ii * 8 + p, ii * 8:(ii + 1) * 8] = 1.0

    # dmask[(n'*8+b'), (n*16+d)] = (n' == n);  sel8[(n'*8+b'), b] = (b' == b)
    dmask = np.zeros((80, ND), dtype=np.float32)
    sel8 = np.zeros((80, 8), dtype=np.float32)
    for n in range(N_CAP):
        for b in range(8):
            dmask[n * 8 + b, n * D_CAP:(n + 1) * D_CAP] = 1.0
            sel8[n * 8 + b, b] = 1.0
    return {"w2": w2, "mask0": mask0, "maskb": maskb, "e8": e8,
            "xmask": xmask, "dmask": dmask, "sel8": sel8}


_NC_CACHE = {}
LAST_RESULT = None  # BassKernelResults of the most recent kernel() call


def _get_nc(I_dim=I_FULL):
    if I_dim not in _NC_CACHE:
        _NC_CACHE[I_dim] = build_nc(I_dim)
    return _NC_CACHE[I_dim]


def kernel(inputs: np.ndarray, W: np.ndarray, trace: bool = False) -> np.ndarray:
    global LAST_RESULT
    from concourse.bass_utils import run_bass_kernel_spmd

    inputs = np.asarray(inputs, dtype=np.float32)
    W = np.asarray(W, dtype=np.float32)
    B, I_dim, _ = inputs.shape

    nc = _get_nc(I_dim)
    shared = prep_shared_inputs(W)

    in_maps = []
    bs = B // N_CORES
    for c in range(N_CORES):
        m = dict(shared)
        m.update(prep_core_inputs(inputs[c * bs:(c + 1) * bs], I_dim))
        in_maps.append(m)

    res = run_bass_kernel_spmd(nc, in_maps, list(range(N_CORES)), trace=trace)
    LAST_RESULT = res
    outs = [res.results[c]["out"] for c in range(N_CORES)]
    v = np.concatenate(outs, axis=0)          # [64, 160]
    v = v.reshape(B, N_CAP, D_CAP)[:, :, None, :]   # [64, 10, 1, 16]
    return v.astype(np.float32)
